# revision 1
# baseline (speedup 1.0000x reference)
"""Trainium2 Bass kernel for the 4-layer sum/product circuit (segment_reduce).

Strategy: shard batch (4096) across 8 cores (512 each), zero communication.
Batch-major SBUF layout: partition p = batch%128, d=4 packed sub-columns
(s = batch//128), single pass covering all 512 batch per core. Gathers are
free-axis ap_gather on GPSIMD (command-bound ~30ns/index, so d=4 packs the
whole batch into each index); indices are baked per 16-partition group.
Sums are in-place DVE accumulates, exp/log on ACT; 256-index chunks with a
double-buffered gather pool overlap DVE/ACT under the Q7 gathers. Input
encoding (log1mexp interleave) and both transposes are host-side.
"""

import math
import numpy as np
from contextlib import ExitStack

import concourse.bacc as bacc
import concourse.tile as tile
from concourse import bass, mybir
from concourse import library_config
from concourse.bass_utils import run_bass_kernel_spmd

N_CORES = 8
B = 4096
BPC = B // N_CORES          # 512 batch per core
D = 4                       # packed sub-columns per gather word group
NPASS = 1                   # single pass: all 512 batch per core
PB = 128 * D                # batch per pass
CHUNK_IDX = 256             # gather indices per ap_gather call

N_XENC = 2050
N_L1 = 8192
N_L2 = 4096
N_L3 = 8192
N_OUT = 2048


def _wrap_idx(flat_idx: np.ndarray) -> np.ndarray:
    """[Q] int -> [128, Q//16] int16 wrapped per 16 partitions, replicated x8."""
    q = flat_idx.shape[0]
    assert q % 16 == 0
    w = flat_idx.reshape(q // 16, 16).T.astype(np.int16)  # [16, Q/16]
    return np.tile(w, (8, 1))  # [128, Q/16]


def _log1mexp(x):
    # match reference (Maechler 2012) in f32
    x = x.astype(np.float32)
    with np.errstate(divide="ignore", invalid="ignore"):
        a = np.log(-np.expm1(x)).astype(np.float32)
        b = np.log1p(-np.exp(x)).astype(np.float32)
    return np.where(x > -math.log(2.0), a, b).astype(np.float32)


def _layer(nc, tc, gpool, t1pool, idx_tile, src_tile, dst_tile,
           n_src, n_out, k, act):
    """dst[:, i*2+s] = act(sum_j src[:, idx[i*k+j]*2+s]) for i in [0,n_out)."""
    fp = mybir.dt.float32
    n_chunks = (n_out * k) // CHUNK_IDX
    outs_per_chunk = CHUNK_IDX // k
    for ci in range(n_chunks):
        g = gpool.tile([128, CHUNK_IDX * D], fp, tag="g")
        nc.gpsimd.ap_gather(
            g[:],
            src_tile[:],
            idx_tile[:, ci * (CHUNK_IDX // 16):(ci + 1) * (CHUNK_IDX // 16)],
            channels=128,
            num_elems=n_src,
            d=D,
            num_idxs=CHUNK_IDX,
        )
        # in-place accumulate over k legs; word layout: (i*k + j)*D + s
        dst_words = outs_per_chunk * D
        acc = t1pool.tile([128, dst_words], fp, tag="acc")
        ga = g[:].rearrange("p (i k d) -> p i k d", k=k, d=D)
        av = acc[:].rearrange("p (i d) -> p i d", d=D)
        nc.vector.tensor_tensor(av, ga[:, :, 0, :], ga[:, :, 1, :],
                                mybir.AluOpType.add)
        for j in range(2, k):
            nc.vector.tensor_tensor(av, av, ga[:, :, j, :],
                                    mybir.AluOpType.add)
        nc.scalar.activation(
            dst_tile[:, ci * dst_words:(ci + 1) * dst_words],
            acc[:],
            act,
        )


def _build(nc):
    fp = mybir.dt.float32
    i16 = mybir.dt.int16
    xenc_d = nc.dram_tensor("xenc", [NPASS, 128, N_XENC * D], fp,
                            kind="ExternalInput").ap()
    idx_d = {}
    for name, q in (("g1", N_L1 * 4), ("g2", N_L2 * 8), ("g3", N_L3 * 4),
                    ("g4", N_OUT * 8)):
        idx_d[name] = nc.dram_tensor(f"{name}idx", [128, q // 16], i16,
                                     kind="ExternalInput").ap()
    out_d = nc.dram_tensor("out_t", [NPASS, 128, N_OUT * D], fp,
                           kind="ExternalOutput").ap()

    with tile.TileContext(nc) as tc, ExitStack() as ctx:
        nc.gpsimd.load_library(library_config.ap_gather)
        idxp = ctx.enter_context(tc.tile_pool(name="idxp", bufs=1))
        bigp = ctx.enter_context(tc.tile_pool(name="bigp", bufs=1))
        smallp = ctx.enter_context(tc.tile_pool(name="smallp", bufs=1))
        gpool = ctx.enter_context(tc.tile_pool(name="gpool", bufs=2))
        t1pool = ctx.enter_context(tc.tile_pool(name="t1pool", bufs=1))

        def load_idx(name):
            t = idxp.tile(list(idx_d[name].shape), i16, tag="idx")
            nc.sync.dma_start(t[:], idx_d[name][:])
            return t
        idx_t = {}

        for ps in range(NPASS):
            xenc = smallp.tile([128, N_XENC * D], fp, tag="small")
            nc.sync.dma_start(xenc[:], xenc_d[ps])
            e1 = bigp.tile([128, N_L1 * D], fp, tag="big")
            _layer(nc, tc, gpool, t1pool, load_idx("g1"), xenc, e1,
                   N_XENC, N_L1, 4, mybir.ActivationFunctionType.Exp)
            l2 = smallp.tile([128, N_L2 * D], fp, tag="small")
            _layer(nc, tc, gpool, t1pool, load_idx("g2"), e1, l2,
                   N_L1, N_L2, 8, mybir.ActivationFunctionType.Ln)
            e3 = bigp.tile([128, N_L3 * D], fp, tag="big")
            _layer(nc, tc, gpool, t1pool, load_idx("g3"), l2, e3,
                   N_L2, N_L3, 4, mybir.ActivationFunctionType.Exp)
            outt = smallp.tile([128, N_OUT * D], fp, tag="small")
            _layer(nc, tc, gpool, t1pool, load_idx("g4"), e3, outt,
                   N_L3, N_OUT, 8, mybir.ActivationFunctionType.Ln)
            nc.sync.dma_start(out_d[ps], outt[:])
    nc.compile()
    return nc


_CACHED_NC = None
_LAST_IN_MAPS = None


def kernel(pos, idx0, idx1, idx2, idx3):
    global _CACHED_NC
    pos = np.asarray(pos, dtype=np.float32)
    in_dtypes = {k: np.asarray(v).dtype for k, v in
                 (("idx0", idx0), ("idx1", idx1), ("idx2", idx2), ("idx3", idx3))}

    # host-side input encoding: x_enc [2050, 4096]
    neg = _log1mexp(pos)
    n, b = pos.shape
    xenc = np.zeros((2 * n + 2, b), np.float32)
    xenc[1] = 0.0
    xenc[2::2] = pos
    xenc[3::2] = neg
    # row 0 is -inf in the reference but never gathered (idx0 >= 1); keep 0.

    idx_maps = {
        "g1idx": _wrap_idx(np.asarray(idx0).reshape(-1)),
        "g2idx": _wrap_idx(np.asarray(idx1).reshape(-1)),
        "g3idx": _wrap_idx(np.asarray(idx2).reshape(-1)),
        "g4idx": _wrap_idx(np.asarray(idx3).reshape(-1)),
    }

    if _CACHED_NC is None:
        _CACHED_NC = _build(bacc.Bacc("TRN2", target_bir_lowering=False,
                                      debug=False))
    nc = _CACHED_NC

    in_maps = []
    for c in range(N_CORES):
        sl = xenc[:, c * BPC:(c + 1) * BPC]  # [2050, 512]
        # [1, p, node*4+s]: batch = p + 128*s
        arr = sl.reshape(N_XENC, D, 128).transpose(2, 0, 1)
        in_maps.append({"xenc": np.ascontiguousarray(
            arr.reshape(NPASS, 128, N_XENC * D)), **idx_maps})

    global _LAST_IN_MAPS
    _LAST_IN_MAPS = in_maps
    res = run_bass_kernel_spmd(nc, in_maps, list(range(N_CORES)))
    out = np.empty((N_OUT, B), np.float32)
    for c in range(N_CORES):
        ot = res.results[c]["out_t"].reshape(128, N_OUT, D)
        # batch = c*512 + s*128 + p
        out[:, c * BPC:(c + 1) * BPC] = ot.transpose(1, 2, 0).reshape(N_OUT, BPC)
    return out



# revision 2
# speedup vs baseline: 1.1263x; 1.1263x over previous
"""Trainium2 Bass kernel for the 4-layer sum/product circuit (segment_reduce).

Strategy: shard batch (4096) across 8 cores (512 each), zero communication.
Batch-major SBUF layout: partition p = batch%128, d=4 packed sub-columns
(s = batch//128). All data bf16 on-chip (rel err ~1e-2 vs 2e-2 gate, host
simulated): halves gather bytes and doubles DVE add throughput (2x_1P mode
needs 16-bit dtype + unit stride).

Per layer, gathers run on GPSIMD in 2048-index chunks whose index stream is
leg-major (leg j of all chunk outputs contiguous), so the k-leg reduction is
a log2(k)-level tree of fully contiguous DVE adds. ACT applies exp/ln per
chunk into the layer output tile. Deep pools (gather bufs=3, tree bufs=2)
keep GPSIMD/DVE/ACT pipelined across chunks; the baseline's bufs=1 acc pool
serialized every chunk behind the previous chunk's activation (3.8ms wall on
1.2ms of engine work).
"""

import math
import numpy as np
from contextlib import ExitStack

import ml_dtypes

import concourse.bacc as bacc
import concourse.tile as tile
from concourse import bass, mybir
from concourse import library_config
from concourse.bass_utils import run_bass_kernel_spmd

N_CORES = 8
B = 4096
BPC = B // N_CORES          # 512 batch per core
D = 4                       # packed sub-columns per gather word group
CHUNK = 2048                # gather indices per ap_gather call

N_XENC = 2050
N_L1 = 8192
N_L2 = 4096
N_L3 = 8192
N_OUT = 2048

LAYERS = [
    # (idx_name, n_src, n_out, k, act)
    ("g1", N_XENC, N_L1, 4, "Exp"),
    ("g2", N_L1, N_L2, 8, "Ln"),
    ("g3", N_L2, N_L3, 4, "Exp"),
    ("g4", N_L3, N_OUT, 8, "Ln"),
]


def _wrap_idx(flat_idx: np.ndarray) -> np.ndarray:
    """[Q] int -> [128, Q//16] int16 wrapped per 16 partitions, replicated x8."""
    q = flat_idx.shape[0]
    assert q % 16 == 0
    w = flat_idx.reshape(q // 16, 16).T.astype(np.int16)  # [16, Q/16]
    return np.tile(w, (8, 1))  # [128, Q/16]


def _legmajor(idx: np.ndarray, k: int) -> np.ndarray:
    """[n_out, k] -> flat stream with leg-major order inside each CHUNK."""
    opc = CHUNK // k
    n_out = idx.shape[0]
    # [n_chunks, opc, k] -> [n_chunks, k, opc]
    return np.ascontiguousarray(
        idx.reshape(n_out // opc, opc, k).transpose(0, 2, 1)).reshape(-1)


def _log1mexp(x):
    # match reference (Maechler 2012) in f32
    x = x.astype(np.float32)
    with np.errstate(divide="ignore", invalid="ignore"):
        a = np.log(-np.expm1(x)).astype(np.float32)
        b = np.log1p(-np.exp(x)).astype(np.float32)
    return np.where(x > -math.log(2.0), a, b).astype(np.float32)


def _layer(nc, gpool, tpool, idx_tile, src_tile, dst_tile, n_src, n_out, k, act):
    """dst[:, o*D+s] = act(sum_j src[:, idx[o*k+j]*D+s]) for o in [0, n_out)."""
    bf = mybir.dt.bfloat16
    opc = CHUNK // k
    W = opc * D                     # words per leg block
    n_chunks = (n_out * k) // CHUNK
    for ci in range(n_chunks):
        g = gpool.tile([128, CHUNK * D], bf, tag="g")
        nc.gpsimd.ap_gather(
            g[:],
            src_tile[:],
            idx_tile[:, ci * (CHUNK // 16):(ci + 1) * (CHUNK // 16)],
            channels=128,
            num_elems=n_src,
            d=D,
            num_idxs=CHUNK,
        )
        tr = tpool.tile([128, (k - 1) * W], bf, tag="tree")
        # level 1: pairwise add of contiguous leg blocks from the gather
        for j in range(k // 2):
            nc.vector.tensor_tensor(
                tr[:, j * W:(j + 1) * W],
                g[:, (2 * j) * W:(2 * j + 1) * W],
                g[:, (2 * j + 1) * W:(2 * j + 2) * W],
                mybir.AluOpType.add,
            )
        # deeper levels within tr
        base_in, cnt, base_out = 0, k // 2, k // 2
        while cnt > 1:
            for j in range(cnt // 2):
                nc.vector.tensor_tensor(
                    tr[:, (base_out + j) * W:(base_out + j + 1) * W],
                    tr[:, (base_in + 2 * j) * W:(base_in + 2 * j + 1) * W],
                    tr[:, (base_in + 2 * j + 1) * W:(base_in + 2 * j + 2) * W],
                    mybir.AluOpType.add,
                )
            base_in, base_out, cnt = base_out, base_out + cnt // 2, cnt // 2
        final = tr[:, (k - 2) * W:(k - 1) * W]
        nc.scalar.activation(
            dst_tile[:, ci * W:(ci + 1) * W],
            final,
            getattr(mybir.ActivationFunctionType, act),
        )


def _build(nc):
    bf = mybir.dt.bfloat16
    fp = mybir.dt.float32
    i16 = mybir.dt.int16
    xenc_d = nc.dram_tensor("xenc", [128, N_XENC * D], bf,
                            kind="ExternalInput").ap()
    idx_d = {}
    for name, n_src, n_out, k, act in LAYERS:
        q = n_out * k
        idx_d[name] = nc.dram_tensor(f"{name}idx", [128, q // 16], i16,
                                     kind="ExternalInput").ap()
    out_d = nc.dram_tensor("out_t", [128, N_OUT * D], fp,
                           kind="ExternalOutput").ap()

    with tile.TileContext(nc) as tc, ExitStack() as ctx:
        nc.gpsimd.load_library(library_config.ap_gather)
        idxp = ctx.enter_context(tc.tile_pool(name="idxp", bufs=1))
        bigp = ctx.enter_context(tc.tile_pool(name="bigp", bufs=1))
        smallp = ctx.enter_context(tc.tile_pool(name="smallp", bufs=1))
        gpool = ctx.enter_context(tc.tile_pool(name="gpool", bufs=3))
        tpool = ctx.enter_context(tc.tile_pool(name="tpool", bufs=2))

        idx_t = {}
        for name, n_src, n_out, k, act in LAYERS:
            t = idxp.tile(list(idx_d[name].shape), i16, tag=name)
            nc.sync.dma_start(t[:], idx_d[name][:])
            idx_t[name] = t

        xenc = smallp.tile([128, N_XENC * D], bf, tag="small")
        nc.sync.dma_start(xenc[:], xenc_d[:])

        e1 = bigp.tile([128, N_L1 * D], bf, tag="big")
        _layer(nc, gpool, tpool, idx_t["g1"], xenc, e1, N_XENC, N_L1, 4, "Exp")
        l2 = smallp.tile([128, N_L2 * D], bf, tag="small")
        _layer(nc, gpool, tpool, idx_t["g2"], e1, l2, N_L1, N_L2, 8, "Ln")
        e3 = bigp.tile([128, N_L3 * D], bf, tag="big")
        _layer(nc, gpool, tpool, idx_t["g3"], l2, e3, N_L2, N_L3, 4, "Exp")
        outt = smallp.tile([128, N_OUT * D], fp, tag="small")
        _layer(nc, gpool, tpool, idx_t["g4"], e3, outt, N_L3, N_OUT, 8, "Ln")
        nc.sync.dma_start(out_d[:], outt[:])
    nc.compile()
    return nc


_CACHED_NC = None
_LAST_IN_MAPS = None


def kernel(pos, idx0, idx1, idx2, idx3):
    global _CACHED_NC, _LAST_IN_MAPS
    pos = np.asarray(pos, dtype=np.float32)

    # host-side input encoding: x_enc [2050, 4096]
    neg = _log1mexp(pos)
    n, b = pos.shape
    xenc = np.zeros((2 * n + 2, b), np.float32)
    xenc[1] = 0.0
    xenc[2::2] = pos
    xenc[3::2] = neg
    # row 0 is -inf in the reference but never gathered (idx0 >= 1); keep 0.

    idx_arrs = {"g1": idx0, "g2": idx1, "g3": idx2, "g4": idx3}
    idx_maps = {}
    for name, n_src, n_out, k, act in LAYERS:
        flat = _legmajor(np.asarray(idx_arrs[name]).reshape(n_out, k), k)
        idx_maps[f"{name}idx"] = _wrap_idx(flat)

    if _CACHED_NC is None:
        _CACHED_NC = _build(bacc.Bacc("TRN2", target_bir_lowering=False,
                                      debug=False))
    nc = _CACHED_NC

    in_maps = []
    for c in range(N_CORES):
        sl = xenc[:, c * BPC:(c + 1) * BPC]  # [2050, 512]
        # [p, node*4+s]: batch = p + 128*s
        arr = sl.reshape(N_XENC, D, 128).transpose(2, 0, 1)
        in_maps.append({"xenc": np.ascontiguousarray(
            arr.reshape(128, N_XENC * D)).astype(ml_dtypes.bfloat16),
            **idx_maps})

    _LAST_IN_MAPS = in_maps
    res = run_bass_kernel_spmd(nc, in_maps, list(range(N_CORES)))
    out = np.empty((N_OUT, B), np.float32)
    for c in range(N_CORES):
        ot = res.results[c]["out_t"].reshape(128, N_OUT, D)
        # batch = c*512 + s*128 + p
        out[:, c * BPC:(c + 1) * BPC] = ot.transpose(1, 2, 0).reshape(N_OUT, BPC)
    return out
